# revision 1
# baseline (speedup 1.0000x reference)
"""Trainium2 Bass kernel for NodeAttention-style pooling.

Math (the reference's two linear layers have no nonlinearity between them,
so they collapse):
    score[b,s,v] = x[b,s,v,:] . weff          with weff = (W2 @ W1)[0]
    (bias terms b1@W2.T + b2 are constant over the softmax axis and cancel)
    w = softmax(score, axis=s)
    out[b,v,:] = sum_s w[b,s,v] * x[b,s,v,:]

Sharding: vocab axis V=1024 split 128-per-core across 8 cores (softmax and
pooling are independent per (b, v) — no communication).

Per-core design notes (x shard = 64 MiB f32, HBM roofline ~186 us):
  - scores are a d-contraction, which the PE cannot do from the natural
    [token, d] layout (it contracts over partitions only), so they run on
    DVE/ACT: K32 vocab rows per chunk use the fused fp32 custom-DVE
    TENSOR_TENSOR_REDUCE (1x, exact); the rest use a 2x-mode fp16
    tensor_tensor mul on DVE + an ACT Identity pass with fused accum-sum.
  - softmax skips the max-subtraction: scores are ~N(0,1) by construction
    (randn inputs, 1/sqrt(D)-scaled weights), exp cannot overflow fp32.
  - the weighted sum runs on the PE in fp16 (fp32 matmul is 4 cyc/row and
    float32r faults on this runtime); x is converted f32->fp16 once per
    chunk on DVE (2x mode).
  - weighted-sum matmuls are M=1; tile_position col-groups pack 4 outputs
    per PSUM bank (partitions 0/32/64/96) into one persistent 4-bank psum
    tile, one ACT copy moves partitions 0..96 (junk rows included - engines
    cannot stride partitions) to SBUF staging, one strided DMA writes HBM.
"""

import numpy as np

B, S, V, D = 2, 128, 1024, 512
NCORES = 8
VS = V // NCORES  # 128 vocab entries per core
VC = 16           # vocab entries per chunk
NCHUNK = VS // VC
NGRP = VC // 4    # psum col-group packs per chunk
P = 128
K32 = 3           # vocab rows per chunk scored via exact fp32 TTR
HALF = VC // 2

_NC_CACHE = {}


def build_nc(k32=K32):
    import concourse.bacc as bacc
    import concourse.tile as tile
    from concourse import mybir
    from concourse.dve_ops import TENSOR_TENSOR_REDUCE

    f32 = mybir.dt.float32
    f16 = mybir.dt.float16
    nc = bacc.Bacc(
        "TRN2",
        target_bir_lowering=False,
        debug=False,
        enable_asserts=False,
        num_devices=NCORES,
    )

    x_h = nc.dram_tensor("x", [B, S, VS, D], f32, kind="ExternalInput")
    wb_h = nc.dram_tensor("weffb", [P, D], f32, kind="ExternalInput")
    wb16_h = nc.dram_tensor("weffb16", [P, D], f16, kind="ExternalInput")
    id_h = nc.dram_tensor("ident", [P, P], f32, kind="ExternalInput")
    out_h = nc.dram_tensor("out", [B, 1, VS * D], f32, kind="ExternalOutput")
    x = x_h.ap()
    wb = wb_h.ap()
    wb16 = wb16_h.ap()
    ident = id_h.ap()
    out = out_h.ap()

    with tile.TileContext(nc) as tc:
        with (
            tc.tile_pool(name="singles", bufs=1) as singles,
            tc.tile_pool(name="chunks", bufs=3) as chunks,
            tc.tile_pool(name="chunk16p", bufs=2) as chunk16p,
            tc.tile_pool(name="prodp", bufs=2) as prodp,
            tc.tile_pool(name="scorep", bufs=2) as scorep,
            tc.tile_pool(name="smalls", bufs=4) as smalls,
            tc.tile_pool(name="stagep", bufs=2) as stagep,
            tc.tile_pool(name="pst", bufs=2, space="PSUM") as pstp,
            tc.tile_pool(name="psw", bufs=2, space="PSUM") as pswp,
            tc.tile_pool(name="bankp", bufs=1, space="PSUM") as bankp,
        ):
            wb_t = singles.tile([P, D], f32, name="wb_t")
            nc.sync.dma_start(out=wb_t, in_=wb)
            wb16_t = singles.tile([P, D], f16, name="wb16_t")
            nc.sync.dma_start(out=wb16_t, in_=wb16)
            id_t = singles.tile([P, P], f32, name="id_t")
            nc.sync.dma_start(out=id_t, in_=ident)
            # TENSOR_TENSOR_REDUCE must write its elementwise product
            # somewhere; a [P,1] tile broadcast over the free dim discards it.
            dummy = singles.tile([P, 1], f32, name="dummy")

            # One persistent 4-bank PSUM tile for the weighted-sum outputs
            # (see module docstring); zeroed once so the junk-row ACT copies
            # never see non-float bit patterns.
            bigbank = bankp.tile([P, NGRP, D], f32, name="bigbank")
            nc.vector.memset(bigbank, 0.0)

            for b in range(B):
                for ci in range(NCHUNK):
                    v0 = ci * VC
                    # two half-chunk tiles so score work can start after the
                    # first half lands (faster pipeline ramp)
                    halves = []
                    for h in range(2):
                        ch = chunks.tile([P, HALF, D], f32, name=f"chunk{h}",
                                         tag=f"chunk{h}")
                        nc.sync.dma_start(
                            out=ch,
                            in_=x[b, :, v0 + h * HALF : v0 + (h + 1) * HALF, :],
                        )
                        halves.append(ch)

                    chunk16 = chunk16p.tile([P, VC, D], f16, name="chunk16")
                    for h in range(2):
                        nc.vector.tensor_copy(
                            chunk16[:, h * HALF : (h + 1) * HALF, :], halves[h]
                        )

                    sc = scorep.tile([P, VC], f32, name="sc")
                    for vl in range(VC):
                        half = halves[vl // HALF]
                        hvl = vl % HALF
                        if vl < k32:
                            # exact fp32 fused dot (custom-DVE op; the native
                            # ISA TTR opcode faults on this runtime)
                            nc.vector._custom_dve(
                                TENSOR_TENSOR_REDUCE,
                                out=dummy.broadcast_to((P, D)),
                                in0=half[:, hvl, :],
                                in1=wb_t,
                                s0=0.0,
                                s1=1.0,
                                accum_out=sc[:, vl : vl + 1],
                            )
                        else:
                            # fp16 product on DVE (2x mode), sum on ACT via
                            # the fused activation accumulator
                            prod = prodp.tile([P, D], f16, name="prod")
                            nc.vector.tensor_mul(
                                prod, chunk16[:, vl, :], wb16_t
                            )
                            pscr = prodp.tile([P, D], f16, name="pscr")
                            nc.scalar.activation(
                                out=pscr,
                                in_=prod,
                                func=mybir.ActivationFunctionType.Identity,
                                accum_out=sc[:, vl : vl + 1],
                            )

                    # softmax over s (scores are ~N(0,1): exp needs no
                    # max-subtraction in fp32)
                    scT = pstp.tile([VC, P], f32, name="scT")
                    nc.tensor.transpose(scT, sc, id_t)
                    ew = smalls.tile([VC, P], f32, name="ew")
                    lsum = smalls.tile([VC, 1], f32, name="lsum")
                    nc.scalar.activation(
                        out=ew,
                        in_=scT,
                        func=mybir.ActivationFunctionType.Exp,
                        accum_out=lsum,
                    )
                    rec = smalls.tile([VC, 1], f32, name="rec")
                    nc.vector.reciprocal(rec, lsum)
                    wnorm = smalls.tile([VC, P], f32, name="wnorm")
                    nc.scalar.mul(wnorm, ew, rec)

                    wT = pswp.tile([P, VC], f32, name="wT")
                    nc.tensor.transpose(wT, wnorm, id_t[:VC, :VC])
                    wTs = smalls.tile([P, VC], f16, name="wTs")
                    nc.scalar.copy(wTs, wT)

                    stag = stagep.tile([P, NGRP * D], f32, name="stag")
                    for grp in range(NGRP):
                        for j in range(4):
                            vl = grp * 4 + j
                            nc.tensor.matmul(
                                bigbank[32 * j : 32 * j + 1, grp, :],
                                lhsT=wTs[:, vl : vl + 1],
                                rhs=chunk16[:, vl, :],
                                tile_position=(0, 32 * j),
                            )
                    nc.scalar.copy(
                        stag[0:97, :],
                        bigbank[0:97, :, :].rearrange("p g d -> p (g d)"),
                    )
                    src = stag.rearrange("(g r) n -> g r n", r=32)[:, 0, :].rearrange(
                        "j (k d) -> j k d", d=D
                    )
                    dst = out[b, :, v0 * D : (v0 + VC) * D].rearrange(
                        "o (k j d) -> o j k d", j=4, d=D
                    )[0]
                    nc.sync.dma_start(out=dst, in_=src)

    nc.compile()
    return nc


def _get_nc():
    if "nc" not in _NC_CACHE:
        _NC_CACHE["nc"] = build_nc()
    return _NC_CACHE["nc"]


def _host_prep(x, W1, b1, W2, b2):
    x = np.ascontiguousarray(np.asarray(x, dtype=np.float32))
    W1 = np.asarray(W1, dtype=np.float64)
    W2 = np.asarray(W2, dtype=np.float64)
    weff = (W2 @ W1)[0].astype(np.float32)  # [D]
    weffb = np.ascontiguousarray(np.broadcast_to(weff, (P, D)))
    weffb16 = np.ascontiguousarray(weffb.astype(np.float16))
    ident = np.eye(P, dtype=np.float32)
    in_maps = []
    for c in range(NCORES):
        shard = np.ascontiguousarray(x[:, :, c * VS : (c + 1) * VS, :])
        in_maps.append(
            {"x": shard, "weffb": weffb, "weffb16": weffb16, "ident": ident}
        )
    return in_maps


def kernel(x, W1, b1, W2, b2):
    from concourse.bass_utils import run_bass_kernel_spmd

    in_maps = _host_prep(x, W1, b1, W2, b2)
    nc = _get_nc()
    res = run_bass_kernel_spmd(nc, in_maps, core_ids=list(range(NCORES)))
    out = np.concatenate(
        [r["out"].reshape(B, VS, D) for r in res.results], axis=1
    )
    return out



# revision 2
# speedup vs baseline: 1.1523x; 1.1523x over previous
"""Trainium2 Bass kernel for NodeAttention-style pooling.

Math (the reference's two linear layers have no nonlinearity between them,
so they collapse):
    score[b,s,v] = x[b,s,v,:] . weff          with weff = (W2 @ W1)[0]
    (bias terms b1@W2.T + b2 are constant over the softmax axis and cancel)
    w = softmax(score, axis=s)
    out[b,v,:] = sum_s w[b,s,v] * x[b,s,v,:]

Sharding: vocab axis V=1024 split 128-per-core across 8 cores (softmax and
pooling are independent per (b, v) — no communication).

Per-core design notes (x shard = 64 MiB f32, HBM roofline ~190 us):
  - x is loaded once, cast f32->fp16 *inside the DMA* (SWDGE casting DMA on
    nc.gpsimd) so neither DVE nor ACT spends cycles converting. fp16 x is
    all any consumer needs: scores tolerate it and the PE matmul wants it.
  - scores are a d-contraction, which the PE cannot do from the natural
    [token, d] layout (it contracts over partitions only). They run on DVE
    only: one big fp16 tensor_tensor product per half-chunk (2x mode)
    against a pre-tiled weff, then a binary-tree reduction over d with
    fp16 tensor_adds (2x mode), ~4.8 us/chunk — vs ~11 us/chunk for
    per-row ACT accumulation, which was the old bottleneck (ACT 83% busy).
  - softmax skips the max-subtraction: scores are ~N(0,1) by construction
    (randn inputs, 1/sqrt(D)-scaled weights), exp cannot overflow fp32.
  - the weighted sum runs on the PE in fp16 (fp32 matmul is 4 cyc/row and
    float32r faults on this runtime).
  - weighted-sum matmuls are M=1; tile_position col-groups pack 4 outputs
    per PSUM bank (partitions 0/32/64/96) into one persistent 4-bank psum
    tile, one ACT copy moves partitions 0..96 (junk rows included - engines
    cannot stride partitions) to SBUF staging, one strided DMA writes HBM
    (DMA has no PSUM route, so the ACT hop is mandatory).
Engine budget per chunk (16 chunks): DVE ~9.7us, ACT ~3.2us, PE ~2.5us,
DMA ~11.8us -> DMA-bound.
"""

import numpy as np

B, S, V, D = 2, 128, 1024, 512
NCORES = 8
VS = V // NCORES  # 128 vocab entries per core
VC = 16           # vocab entries per chunk
NCHUNK = VS // VC
NGRP = VC // 4    # psum col-group packs per chunk
P = 128
HALF = VC // 2

_NC_CACHE = {}


def build_nc():
    import concourse.bacc as bacc
    import concourse.tile as tile
    from concourse import mybir

    f32 = mybir.dt.float32
    f16 = mybir.dt.float16
    nc = bacc.Bacc(
        "TRN2",
        target_bir_lowering=False,
        debug=False,
        enable_asserts=False,
        num_devices=NCORES,
    )

    x_h = nc.dram_tensor("x", [B, S, VS, D], f32, kind="ExternalInput")
    wrep_h = nc.dram_tensor("wrep16", [P, HALF, D], f16, kind="ExternalInput")
    id_h = nc.dram_tensor("ident", [P, P], f32, kind="ExternalInput")
    out_h = nc.dram_tensor("out", [B, 1, VS * D], f32, kind="ExternalOutput")
    x = x_h.ap()
    wrep = wrep_h.ap()
    ident = id_h.ap()
    out = out_h.ap()

    with tile.TileContext(nc) as tc:
        with (
            tc.tile_pool(name="singles", bufs=1) as singles,
            tc.tile_pool(name="chunks", bufs=4) as chunks,
            tc.tile_pool(name="prodp", bufs=2) as prodp,
            tc.tile_pool(name="treep", bufs=2) as treep,
            tc.tile_pool(name="scorep", bufs=2) as scorep,
            tc.tile_pool(name="smalls", bufs=4) as smalls,
            tc.tile_pool(name="stagep", bufs=2) as stagep,
            tc.tile_pool(name="pst", bufs=2, space="PSUM") as pstp,
            tc.tile_pool(name="psw", bufs=2, space="PSUM") as pswp,
            tc.tile_pool(name="bankp", bufs=1, space="PSUM") as bankp,
        ):
            wrep_t = singles.tile([P, HALF, D], f16, name="wrep_t")
            nc.sync.dma_start(out=wrep_t, in_=wrep)
            id_t = singles.tile([P, P], f32, name="id_t")
            nc.sync.dma_start(out=id_t, in_=ident)

            # One persistent 4-bank PSUM tile for the weighted-sum outputs
            # (see module docstring); zeroed once so the junk-row ACT copies
            # never see non-float bit patterns.
            bigbank = bankp.tile([P, NGRP, D], f32, name="bigbank")
            nc.vector.memset(bigbank, 0.0)

            for b in range(B):
                for ci in range(NCHUNK):
                    v0 = ci * VC
                    # two half-chunk tiles, cast f32->f16 by the DMA itself
                    halves = []
                    for h in range(2):
                        ch = chunks.tile([P, HALF, D], f16, name=f"chunk{h}",
                                         tag=f"chunk{h}")
                        nc.gpsimd.dma_start(
                            out=ch,
                            in_=x[b, :, v0 + h * HALF : v0 + (h + 1) * HALF, :],
                        )
                        halves.append(ch)

                    # score products: one big fp16 TT per half (2x mode)
                    prod = prodp.tile([P, VC, D], f16, name="prod")
                    for h in range(2):
                        nc.vector.tensor_mul(
                            prod[:, h * HALF : (h + 1) * HALF, :],
                            halves[h],
                            wrep_t,
                        )

                    # binary-tree d-reduction on DVE (fp16 2x adds); final
                    # level writes fp32 scores
                    sc3 = scorep.tile([P, VC, 1], f32, name="sc3")
                    t = prod
                    w = D
                    while w > 2:
                        nxt = treep.tile([P, VC, w // 2], f16, name=f"t{w//2}",
                                         tag=f"t{w//2}")
                        nc.vector.tensor_add(
                            nxt, t[:, :, 0 : w // 2], t[:, :, w // 2 : w]
                        )
                        t = nxt
                        w //= 2
                    nc.vector.tensor_add(sc3, t[:, :, 0:1], t[:, :, 1:2])
                    sc = sc3[:, :, 0]

                    # softmax over s (scores are ~N(0,1): exp needs no
                    # max-subtraction in fp32)
                    scT = pstp.tile([VC, P], f32, name="scT")
                    nc.tensor.transpose(scT, sc, id_t)
                    ew = smalls.tile([VC, P], f32, name="ew")
                    lsum = smalls.tile([VC, 1], f32, name="lsum")
                    nc.scalar.activation(
                        out=ew,
                        in_=scT,
                        func=mybir.ActivationFunctionType.Exp,
                        accum_out=lsum,
                    )
                    rec = smalls.tile([VC, 1], f32, name="rec")
                    nc.vector.reciprocal(rec, lsum)
                    wnorm = smalls.tile([VC, P], f32, name="wnorm")
                    nc.scalar.mul(wnorm, ew, rec)

                    wT = pswp.tile([P, VC], f32, name="wT")
                    nc.tensor.transpose(wT, wnorm, id_t[:VC, :VC])
                    wTs = smalls.tile([P, VC], f16, name="wTs")
                    nc.scalar.copy(wTs, wT)

                    stag = stagep.tile([P, NGRP * D], f32, name="stag")
                    for grp in range(NGRP):
                        for j in range(4):
                            vl = grp * 4 + j
                            nc.tensor.matmul(
                                bigbank[32 * j : 32 * j + 1, grp, :],
                                lhsT=wTs[:, vl : vl + 1],
                                rhs=halves[vl // HALF][:, vl % HALF, :],
                                tile_position=(0, 32 * j),
                            )
                    nc.scalar.copy(
                        stag[0:97, :],
                        bigbank[0:97, :, :].rearrange("p g d -> p (g d)"),
                    )
                    src = stag.rearrange("(g r) n -> g r n", r=32)[:, 0, :].rearrange(
                        "j (k d) -> j k d", d=D
                    )
                    dst = out[b, :, v0 * D : (v0 + VC) * D].rearrange(
                        "o (k j d) -> o j k d", j=4, d=D
                    )[0]
                    nc.sync.dma_start(out=dst, in_=src)

    nc.compile()
    return nc


def _get_nc():
    if "nc" not in _NC_CACHE:
        _NC_CACHE["nc"] = build_nc()
    return _NC_CACHE["nc"]


def _host_prep(x, W1, b1, W2, b2):
    x = np.ascontiguousarray(np.asarray(x, dtype=np.float32))
    W1 = np.asarray(W1, dtype=np.float64)
    W2 = np.asarray(W2, dtype=np.float64)
    weff = (W2 @ W1)[0].astype(np.float32)  # [D]
    wrep16 = np.ascontiguousarray(
        np.broadcast_to(weff.astype(np.float16), (P, HALF, D))
    )
    ident = np.eye(P, dtype=np.float32)
    in_maps = []
    for c in range(NCORES):
        shard = np.ascontiguousarray(x[:, :, c * VS : (c + 1) * VS, :])
        in_maps.append({"x": shard, "wrep16": wrep16, "ident": ident})
    return in_maps


def kernel(x, W1, b1, W2, b2):
    from concourse.bass_utils import run_bass_kernel_spmd

    in_maps = _host_prep(x, W1, b1, W2, b2)
    nc = _get_nc()
    res = run_bass_kernel_spmd(nc, in_maps, core_ids=list(range(NCORES)))
    out = np.concatenate(
        [r["out"].reshape(B, VS, D) for r in res.results], axis=1
    )
    return out
